# revision 32
# baseline (speedup 1.0000x reference)
"""Batched KDE kernel for Trainium2 (8 NeuronCores, SPMD).

Problem: out[b, n] = sum_m exp(-||Xq[b,n] - Xf[b,m]||^2 / bw[b])
  with Silverman bandwidth bw[b] from Xf; b=4, n=m=4096, d=32.

Sharding: data-parallel over batch b (4 batches x 2 shards of query rows
= 8 cores). Each core handles n_shard=2048 query rows against the full
m=4096 fit set of its batch.

Device algorithm (per core), raw Bass with manual semaphores:
  - One augmented K=64 matmul computes psum[n, m] = 2*dot - nmu2 directly:
      lhsT rows 0-31 = 2*XQ^T, rows 32-63 = -1
      rhs  rows 0-31 = XF^T,   rows 32-63 = (XF^T)^2
    (fp32r operands -> 1 col/cycle on the PE)
  - ScalarE activation computes exp(psum/bw - nx2/bw) in-place on PSUM
    with a fused per-partition accumulate (accum_out) -> the sum over m.
  - nx2 (query norms) is computed on-device from the raw query rows.
Host does sharding/layout/packing plus the 4 scalar bandwidth values
(the global quantile needs a sort, which is pathological on-device).
"""

import os
import numpy as np

B, N, M, D = 4, 4096, 4096, 32
NCORES = 8
SHARDS_PER_BATCH = NCORES // B  # 2
NSHARD = N // SHARDS_PER_BATCH  # 2048
NT = NSHARD // 128  # 16 n-tiles per core
K = 2 * D  # augmented contraction dim = 64
MCHUNK = 512  # matmul free-dim chunk (one psum bank)
ACT_FD = 2048  # activation free dim (4 psum banks)
NG = NT * (M // ACT_FD)  # 32 matmul/exp groups
CPG = ACT_FD // MCHUNK  # matmuls (psum banks) per group = 4

# matmul input dtype: "fp32" (exact, 4 cyc/row) or "fp32r" (1 cyc/row)
MM_DTYPE = os.environ.get("BASS_KDE_MM", "fp32r")

_cached = {}


def _build_program():
    import concourse.bass as bass
    import concourse.mybir as mybir
    from contextlib import ExitStack

    nc = bass.Bass()
    f32 = mybir.dt.float32
    mm_dt = mybir.dt.float32r if MM_DTYPE == "fp32r" else f32

    xqt = nc.declare_dram_parameter("xqt", [K, NSHARD], mm_dt, isOutput=False)
    xft = nc.declare_dram_parameter("xft", [K, M], mm_dt, isOutput=False)
    # xqn: cols 0..NT*D-1 = raw query rows, col NT*D = 1/bw,
    #      cols NT*D+1 .. NT*D+NT = -1/bw replicated NT times
    XQN_W = NT * D + 1 + NT
    xqn = nc.declare_dram_parameter("xqn", [128, XQN_W], f32, isOutput=False)
    res = nc.declare_dram_parameter("res", [128, NT], f32, isOutput=True)

    with ExitStack() as ctx:
        lhs_sb = ctx.enter_context(nc.sbuf_tensor([K, NSHARD], mm_dt))
        rhs_sb = ctx.enter_context(nc.sbuf_tensor([K, M], mm_dt))
        xqn_sb = ctx.enter_context(nc.sbuf_tensor([128, XQN_W], f32))
        sq = ctx.enter_context(nc.sbuf_tensor([128, NT * D], f32))
        nx2r = ctx.enter_context(nc.sbuf_tensor([128, NT], f32))
        bias_all = ctx.enter_context(nc.sbuf_tensor([128, NT], f32))
        acc = ctx.enter_context(nc.sbuf_tensor([128, NG], f32))
        res_sb = ctx.enter_context(nc.sbuf_tensor([128, NT], f32))
        ps0 = ctx.enter_context(nc.psum_tensor("ps0", [128, ACT_FD], f32))
        ps1 = ctx.enter_context(nc.psum_tensor("ps1", [128, ACT_FD], f32))
        ps = [ps0, ps1]
        sem_xqn = ctx.enter_context(nc.semaphore("sem_xqn"))
        sem_lhs = ctx.enter_context(nc.semaphore("sem_lhs"))
        sem_rhs0 = ctx.enter_context(nc.semaphore("sem_rhs0"))
        sem_rhs1 = ctx.enter_context(nc.semaphore("sem_rhs1"))
        sem_out = ctx.enter_context(nc.semaphore("sem_out"))
        s_dve = ctx.enter_context(nc.semaphore("s_dve"))
        s_act = ctx.enter_context(nc.semaphore("s_act"))
        s_pe = ctx.enter_context(nc.semaphore("s_pe"))
        block = ctx.enter_context(nc.Block())

        scale_pos = xqn_sb[:, NT * D : NT * D + 1]  # 1/bw
        neg_invbw = xqn_sb[:, NT * D + 1 : NT * D + 1 + NT]  # -1/bw x NT

        @block.sync
        def _(sync):
            sync.dma_start(xqn_sb[:], xqn[:]).then_inc(sem_xqn, 16)
            sync.dma_start(lhs_sb[:], xqt[:]).then_inc(sem_lhs, 16)
            sync.dma_start(rhs_sb[:, 0 : M // 2], xft[:, 0 : M // 2]).then_inc(
                sem_rhs0, 16
            )
            sync.dma_start(rhs_sb[:, M // 2 : M], xft[:, M // 2 : M]).then_inc(
                sem_rhs1, 16
            )
            sync.wait_ge(s_dve, 4)
            sync.dma_start(res[:], res_sb[:]).then_inc(sem_out, 16)
            sync.wait_ge(sem_out, 16)

        @block.vector
        def _(vector):
            vector.wait_ge(sem_xqn, 16)
            nc.vector.tensor_tensor(
                sq[:],
                xqn_sb[:, : NT * D],
                xqn_sb[:, : NT * D],
                op=mybir.AluOpType.mult,
            ).then_inc(s_dve, 1)
            vector.wait_ge(s_dve, 1)
            nc.vector.tensor_reduce(
                nx2r[:],
                sq[:].rearrange("p (t d) -> p t d", d=D),
                axis=mybir.AxisListType.X,
                op=mybir.AluOpType.add,
            ).then_inc(s_dve, 1)
            vector.wait_ge(s_dve, 2)
            nc.vector.tensor_tensor(
                bias_all[:], nx2r[:], neg_invbw, op=mybir.AluOpType.mult
            ).then_inc(s_dve, 1)
            vector.wait_ge(s_act, NG)
            nc.vector.tensor_reduce(
                res_sb[:],
                acc[:].rearrange("p (t h) -> p t h", h=M // ACT_FD),
                axis=mybir.AxisListType.X,
                op=mybir.AluOpType.add,
            ).then_inc(s_dve, 1)

        @block.scalar
        def _(scalar):
            scalar.wait_ge(sem_xqn, 16)
            scalar.wait_ge(s_dve, 3)
            for g in range(NG):
                t = g // 2
                scalar.wait_ge(s_pe, CPG * (g + 1))
                nc.scalar.activation(
                    ps[g % 2][:],
                    ps[g % 2][:],
                    mybir.ActivationFunctionType.Exp,
                    bias=bias_all[:, t : t + 1],
                    scale=scale_pos,
                    accum_out=acc[:, g : g + 1],
                ).then_inc(s_act, 1)

        @block.tensor
        def _(tensor):
            tensor.wait_ge(sem_lhs, 16)
            g = 0
            for t in range(NT):
                lhsT = lhs_sb[:, t * 128 : (t + 1) * 128]
                for h in range(M // ACT_FD):
                    if g == 0:
                        tensor.wait_ge(sem_rhs0, 16)
                    elif g == 1:
                        tensor.wait_ge(sem_rhs1, 16)
                    if g >= 2:
                        tensor.wait_ge(s_act, g - 1)
                    for j in range(CPG):
                        m0 = h * ACT_FD + j * MCHUNK
                        nc.tensor.matmul(
                            ps[g % 2][:, j * MCHUNK : (j + 1) * MCHUNK],
                            lhsT,
                            rhs_sb[:, m0 : m0 + MCHUNK],
                            start=True,
                            stop=True,
                        ).then_inc(s_pe, 1)
                    g += 1

    return nc


def _bandwidth_np(X_fit):
    # mirror of reference._bandwidth (Silverman-style)
    b, n, d = X_fit.shape
    flat = np.asarray(X_fit, dtype=np.float64).reshape(-1)
    q = np.quantile(flat, 0.75) - np.quantile(flat, 0.25)
    std = np.std(np.asarray(X_fit, dtype=np.float64).reshape(b, -1), axis=1, ddof=1)
    return (0.9 * np.minimum(std, q / 1.34) / (n**0.2)).astype(np.float32)


def _host_prep(X_query, X_fit):
    X_query = np.asarray(X_query, dtype=np.float32)
    X_fit = np.asarray(X_fit, dtype=np.float32)
    bw = _bandwidth_np(X_fit)  # [B]

    in_maps = []
    for c in range(NCORES):
        b = c // SHARDS_PER_BATCH
        s = c % SHARDS_PER_BATCH
        XQ = X_query[b, s * NSHARD : (s + 1) * NSHARD]  # [2048, 32]
        XF = X_fit[b]  # [4096, 32]

        # permuted queries: tile t / partition p handles query row p*NT + t
        XQp = XQ.reshape(128, NT, D).transpose(1, 0, 2).reshape(NSHARD, D)
        xqt = np.concatenate(
            [2.0 * XQp.T, -np.ones((D, NSHARD), dtype=np.float32)], axis=0
        ).astype(np.float32)  # [64, 2048]
        xft_t = np.ascontiguousarray(XF.T.astype(np.float32))  # [32, 4096]
        xft = np.concatenate([xft_t, xft_t * xft_t], axis=0)  # [64, 4096]

        inv_bw = np.float32(1.0) / bw[b]
        xqn = np.empty((128, NT * D + 1 + NT), dtype=np.float32)
        xqn[:, : NT * D] = XQ.reshape(128, NT * D)
        xqn[:, NT * D] = inv_bw
        xqn[:, NT * D + 1 :] = -inv_bw

        in_maps.append({"xqt": xqt, "xft": xft, "xqn": xqn})
    return in_maps


def _gather(results):
    out = np.empty((B, N), dtype=np.float32)
    for c in range(NCORES):
        b = c // SHARDS_PER_BATCH
        s = c % SHARDS_PER_BATCH
        res = np.asarray(results[c]["res"], dtype=np.float32)  # [128, 16]
        out[b, s * NSHARD : (s + 1) * NSHARD] = res.reshape(NSHARD)
    return out


def kernel(X_query, X_fit):
    from concourse.bass_utils import run_bass_kernel_spmd

    if "nc" not in _cached:
        _cached["nc"] = _build_program()
    nc = _cached["nc"]
    in_maps = _host_prep(X_query, X_fit)
    out = run_bass_kernel_spmd(nc, in_maps, list(range(NCORES)))
    return _gather(out.results)
